# revision 16
# baseline (speedup 1.0000x reference)
"""Trainium2 Bass kernel for DiseaseKnowledgeModule.

Math (per token z in R^d, d=1024, 14 diseases x 2 states):
  score = z @ flat_memory.T / sqrt(d)                    [28]
  p     = softmax over states [:, 1] == sigmoid(s1 - s0) [14]
  mlc   = max over patches of p                          [B, 14]
  R     = p @ M_present                                  [d]
  gate  = sigmoid([z, R] @ gate_w.T + gate_b)            [d]
  z_out = z + gate * R

Device formulation (feature-major / transposed activations):
  sd    = Wdiff contraction: (m1 - m0) . z               one matmul, K=d
  p     = sigmoid(sd / sqrt(d))
  gate^T= sigmoid(Wg1 @ z^T + (M1 @ Wg2^T)^T-fold @ p^T + b)   K=d + K=14
  R^T   = M1-contraction: K=14
  zout^T= z^T + gate^T * R^T

Sharding: data-parallel over batch B=32 across 8 cores (4 batches/core).
Matmuls in bf16 (fp32 PSUM accumulate); residual add in fp32.
"""
import sys
import numpy as np

sys.path.insert(0, "/opt/trn_rl_repo")

import ml_dtypes
import concourse.bass as bass
import concourse.mybir as mybir
from concourse.bass_utils import run_bass_kernel_spmd

F32 = mybir.dt.float32
BF16 = mybir.dt.bfloat16
AX = mybir.AxisListType
ALU = mybir.AluOpType
ACTF = mybir.ActivationFunctionType
NP_BF16 = ml_dtypes.bfloat16

B, S, D = 32, 1024, 1024
ND, NS = 14, 2
CORES = 8
B_LOC = B // CORES            # 4 batches per core
T = B_LOC * S                 # 4096 tokens per core
NTILE = 512                   # tokens per T-tile (one PSUM bank of fp32)
NT = T // NTILE               # 8 T-tiles
KC = D // 128                 # 8 contraction chunks of 128
JC = D // 128                 # 8 output d-tiles of 128
XBUF = 3                      # bf16 x tile buffers
SCALE = 1.0 / float(np.sqrt(np.float32(D)))

_prog_cache = {}


def _op_numbering():
    """Per-engine op indices (1-based cumulative semaphore values)."""
    pe, act, dve = {}, {}, {}
    n = 0
    n += 1; pe[("score", 0)] = n
    for t in range(NT):
        if 2 <= t + 1 < NT:
            n += 1; pe[("score", t + 1)] = n
        for j in range(JC):
            n += 1; pe[("gate", t, j)] = n
            n += 1; pe[("r", t, j)] = n
        if t == 0 and NT > 1:
            n += 1; pe[("score", 1)] = n
    n = 0
    for t in range(NT):
        n += 1; act[("psig", t)] = n
        n += 1; act[("pcast", t)] = n
        for j in range(JC):
            n += 1; act[("gsig", t, j)] = n
    n = 0
    for t in range(NT):
        n += 1; dve[("rmax", t)] = n
        for j in range(JC):
            n += 1; dve[("mul", t, j)] = n
            n += 1; dve[("add", t, j)] = n
    return pe, act, dve


def _build_program():
    from concourse.bass import compact_to_ranges

    pe_i, act_i, dve_i = _op_numbering()
    nc = bass.Bass()

    # A previous NEFF on this core (e.g. an XLA graph) may leave kernel-range
    # semaphores nonzero; our waits use absolute values and assume they start
    # at 0. Mirror the target_bir_lowering=True prologue: clear the whole
    # kernel sem range on gpsimd, then hold every engine at an NRT-level
    # barrier (which does not depend on bass sems) until the clear lands.
    for sem_range in compact_to_ranges(
            [s for s in nc._kernel_sem_range if s not in nc.barrier_sems]):
        nc.gpsimd.dma_reset(sem_range)
        nc.gpsimd.sem_clear(sem_range)
    nc._nrt_pseudo_barrier()

    xb16 = nc.dram_tensor("xb16", [D, T], BF16, kind="ExternalInput")
    xf32 = nc.dram_tensor("xf32", [D, T], F32, kind="ExternalInput")
    wg1T = nc.dram_tensor("wg1T", [D, D], BF16, kind="ExternalInput")
    wdT = nc.dram_tensor("wdT", [D, ND], BF16, kind="ExternalInput")
    wg2pT = nc.dram_tensor("wg2pT", [ND, D], BF16, kind="ExternalInput")
    mpres = nc.dram_tensor("mpres", [ND, D], BF16, kind="ExternalInput")
    biasd = nc.dram_tensor("biasd", [128, JC], F32, kind="ExternalInput")
    zoT = nc.dram_tensor("zoT", [D, T], F32, kind="ExternalOutput")
    mlc8 = nc.dram_tensor("mlc8", [ND, NT], F32, kind="ExternalOutput")

    from contextlib import ExitStack
    with ExitStack() as ctx:
        xb = ctx.enter_context(nc.sbuf_tensor("xb", [128, XBUF * KC * NTILE], BF16))
        xf = ctx.enter_context(nc.sbuf_tensor("xf", [128, 2 * KC * NTILE], F32))
        wg1 = ctx.enter_context(nc.sbuf_tensor("wg1", [128, KC * D], BF16))
        wdiff = ctx.enter_context(nc.sbuf_tensor("wdiff", [128, KC * ND], BF16))
        wg2p = ctx.enter_context(nc.sbuf_tensor("wg2p", [ND, D], BF16))
        mpr = ctx.enter_context(nc.sbuf_tensor("mpr", [ND, D], BF16))
        bias_sb = ctx.enter_context(nc.sbuf_tensor("bias_sb", [128, JC], F32))
        zbias = ctx.enter_context(nc.sbuf_tensor("zbias", [ND, 1], F32))
        p_sb = ctx.enter_context(nc.sbuf_tensor("p_sb", [ND, 2 * NTILE], BF16))
        g_sb = ctx.enter_context(nc.sbuf_tensor("g_sb", [128, 2 * NTILE], F32))
        zo = ctx.enter_context(nc.sbuf_tensor("zo", [128, 2 * JC * NTILE], F32))
        p32_sb = ctx.enter_context(nc.sbuf_tensor("p32_sb", [ND, 2 * NTILE], F32))
        mlc_sb = ctx.enter_context(nc.sbuf_tensor("mlc_sb", [ND, NT], F32))
        sd_ps = ctx.enter_context(nc.psum_tensor("sd_ps", [ND, 2 * NTILE], F32))
        gate_ps = ctx.enter_context(nc.psum_tensor("gate_ps", [128, 2 * NTILE], F32))
        r_ps = ctx.enter_context(nc.psum_tensor("r_ps", [128, 2 * NTILE], F32))
        zsem = ctx.enter_context(nc.semaphore("zsem"))
        wdsem = ctx.enter_context(nc.semaphore("wdsem"))
        bsem = ctx.enter_context(nc.semaphore("bsem"))
        wsem = ctx.enter_context(nc.semaphore("wsem"))
        w1sem = [ctx.enter_context(nc.semaphore(f"w1sem{i}")) for i in range(4)]
        xbsem = [ctx.enter_context(nc.semaphore(f"xbsem{i}"))
                 for i in range(XBUF)]
        xfsem = [ctx.enter_context(nc.semaphore(f"xfsem{i}"))
                 for i in range(2)]
        osem = [ctx.enter_context(nc.semaphore(f"osem{i}"))
                for i in range(2)]
        pe_sem = ctx.enter_context(nc.semaphore("pe_sem"))
        act_sem = ctx.enter_context(nc.semaphore("act_sem"))
        dve_sem = ctx.enter_context(nc.semaphore("dve_sem"))
        block = ctx.enter_context(nc.Block(no_gpsimd_drain=True))

        # ---- slice helpers ----
        def xb_sl(t, k):        # bf16 rhs [128, NTILE] for k-chunk of tile t
            o = (t % XBUF) * KC * NTILE + k * NTILE
            return xb.ap()[:, o:o + NTILE]

        def xb_tile(t):
            o = (t % XBUF) * KC * NTILE
            return xb.ap()[:, o:o + KC * NTILE].rearrange(
                "p (k n) -> p k n", k=KC)

        def xf_sl(t, j):        # f32 residual [128, NTILE] for d-tile j
            o = (t % 2) * KC * NTILE + j * NTILE
            return xf.ap()[:, o:o + NTILE]

        def xf_tile(t):
            o = (t % 2) * KC * NTILE
            return xf.ap()[:, o:o + KC * NTILE].rearrange(
                "p (k n) -> p k n", k=KC)

        def wg1_sl(k, j):
            o = k * D + j * 128
            return wg1.ap()[:, o:o + 128]

        def wd_sl(k):
            o = k * ND
            return wdiff.ap()[:, o:o + ND]

        def p_sl(t):
            o = (t % 2) * NTILE
            return p_sb.ap()[:, o:o + NTILE]

        def p32_sl(t):
            o = (t % 2) * NTILE
            return p32_sb.ap()[:, o:o + NTILE]

        def g_sl(j):
            o = (j % 2) * NTILE
            return g_sb.ap()[:, o:o + NTILE]

        def zo_sl(t, j):
            o = (t % 2) * JC * NTILE + j * NTILE
            return zo.ap()[:, o:o + NTILE]

        def zo_tile(t):
            o = (t % 2) * JC * NTILE
            return zo.ap()[:, o:o + JC * NTILE].rearrange(
                "p (j n) -> p j n", j=JC)

        def sd_sl(t):
            o = (t % 2) * NTILE
            return sd_ps.ap()[:, o:o + NTILE]

        def gate_sl(j):
            o = (j % 2) * NTILE
            return gate_ps.ap()[:, o:o + NTILE]

        def r_sl(j):
            o = (j % 2) * NTILE
            return r_ps.ap()[:, o:o + NTILE]

        # ---- GPSIMD: constants init ----
        @block.gpsimd
        def _(gpsimd):
            nc.gpsimd.memset(zbias.ap(), 0.0).then_inc(zsem, 1)

        # ---- SP: all DMA ----
        @block.sync
        def _(sync):
            xbsrc = xb16.rearrange("(k p) (t n) -> p k t n", p=128, n=NTILE)
            xfsrc = xf32.rearrange("(k p) (t n) -> p k t n", p=128, n=NTILE)

            sync.dma_start(
                wdiff.ap().rearrange("p (k m) -> p k m", k=KC),
                wdT.rearrange("(k p) m -> p k m", p=128),
            ).then_inc(wdsem, 16)
            sync.dma_start(bias_sb.ap(), biasd[:, :]).then_inc(bsem, 16)
            sync.dma_start(xb_tile(0), xbsrc[:, :, 0, :]).then_inc(xbsem[0], 16)
            w1dst = wg1.ap().rearrange("p (k m) -> p k m", k=KC)
            w1src = wg1T.rearrange("(k p) m -> p k m", p=128)
            for q in range(4):
                sync.dma_start(w1dst[:, 2 * q:2 * q + 2, :],
                               w1src[:, 2 * q:2 * q + 2, :]
                               ).then_inc(w1sem[q], 16)
            sync.dma_start(wg2p.ap(), wg2pT[:, :]).then_inc(wsem, 16)
            sync.dma_start(mpr.ap(), mpres[:, :]).then_inc(wsem, 16)
            sync.dma_start(xb_tile(1), xbsrc[:, :, 1, :]).then_inc(xbsem[1], 16)
            sync.dma_start(xf_tile(0), xfsrc[:, :, 0, :]).then_inc(xfsem[0], 16)
            sync.dma_start(xb_tile(2), xbsrc[:, :, 2, :]).then_inc(xbsem[2], 16)
            sync.dma_start(xf_tile(1), xfsrc[:, :, 1, :]).then_inc(xfsem[1], 16)

            zdst = zoT.rearrange("(j p) (t n) -> p j t n", p=128, n=NTILE)
            for t in range(NT - 1):
                sync.wait_ge(dve_sem, dve_i[("add", t, JC - 1)])
                sync.dma_start(zdst[:, :, t, :],
                               zo_tile(t)).then_inc(osem[t % 2], 16)
                if t + XBUF < NT:
                    sync.dma_start(xb_tile(t + XBUF),
                                   xbsrc[:, :, t + XBUF, :]
                                   ).then_inc(xbsem[t % XBUF], 16)
                if t + 2 < NT:
                    sync.dma_start(xf_tile(t + 2),
                                   xfsrc[:, :, t + 2, :]
                                   ).then_inc(xfsem[t % 2], 16)
            # last tile: stream each d-slice out as soon as its add lands,
            # and let the (tiny) mlc transfer fly as early as possible
            tl = NT - 1
            sync.wait_ge(dve_sem, dve_i[("rmax", NT - 1)])
            sync.dma_start(mlc8[:, :], mlc_sb.ap()).then_inc(osem[0], 16)
            for j in range(JC):
                sync.wait_ge(dve_sem, dve_i[("add", tl, j)])
                sync.dma_start(zdst[:, j, tl, :],
                               zo_sl(tl, j)).then_inc(osem[tl % 2], 16)
            # slot0: tiles 0,2,4,6 + mlc = 5 transfers; slot1: 1,3,5 + 8 j
            sync.wait_ge(osem[0], 16 * (NT // 2 + 1))
            sync.wait_ge(osem[1], 16 * (NT // 2 - 1 + JC))

        # ---- PE: all matmuls (bf16 in, fp32 accumulate) ----
        @block.tensor
        def _(tensor):
            def score(t):
                tensor.wait_ge(xbsem[t % XBUF], 16 * (t // XBUF + 1))
                if t >= 2:
                    tensor.wait_ge(act_sem, act_i[("psig", t - 2)])
                for k in range(KC):
                    mm = nc.tensor.matmul(
                        sd_sl(t), wd_sl(k), xb_sl(t, k),
                        start=(k == 0), stop=(k == KC - 1))
                mm.then_inc(pe_sem, 1)

            tensor.wait_ge(wdsem, 16)
            score(0)
            for t in range(NT):
                if 2 <= t + 1 < NT:
                    score(t + 1)
                tensor.wait_ge(act_sem, act_i[("pcast", t)])
                for j in range(JC):
                    # gate_pre_j = sum_k Wg1[k,j]^T x[k] + Wg2p[j]^T p
                    if j >= 2:
                        tensor.wait_ge(act_sem, act_i[("gsig", t, j - 2)])
                    elif t >= 1:
                        tensor.wait_ge(act_sem, act_i[("gsig", t - 1, j + JC - 2)])
                    for k in range(KC):
                        if t == 0 and j == 0:
                            if k % 2 == 0:
                                tensor.wait_ge(w1sem[k // 2], 16)
                            if k == KC - 1:
                                tensor.wait_ge(wsem, 32)
                        nc.tensor.matmul(
                            gate_sl(j), wg1_sl(k, j), xb_sl(t, k),
                            start=(k == 0), stop=False)
                    mm = nc.tensor.matmul(
                        gate_sl(j), wg2p.ap()[:, j * 128:(j + 1) * 128],
                        p_sl(t), start=False, stop=True)
                    mm.then_inc(pe_sem, 1)
                    # R_j = M1[j]^T p
                    if j >= 2:
                        tensor.wait_ge(dve_sem, dve_i[("mul", t, j - 2)])
                    elif t >= 1:
                        tensor.wait_ge(dve_sem, dve_i[("mul", t - 1, j + JC - 2)])
                    mm = nc.tensor.matmul(
                        r_sl(j), mpr.ap()[:, j * 128:(j + 1) * 128],
                        p_sl(t), start=True, stop=True)
                    mm.then_inc(pe_sem, 1)
                if t == 0 and NT > 1:
                    score(1)

        # ---- ACT: sigmoids ----
        @block.scalar
        def _(scalar):
            scalar.wait_ge(zsem, 1)
            for t in range(NT):
                scalar.wait_ge(pe_sem, pe_i[("score", t)])
                if t >= 2:
                    scalar.wait_ge(pe_sem, pe_i[("r", t - 2, JC - 1)])
                    scalar.wait_ge(dve_sem, dve_i[("rmax", t - 2)])
                nc.scalar.activation(
                    p32_sl(t), sd_sl(t), ACTF.Sigmoid, bias=zbias.ap(),
                    scale=SCALE).then_inc(act_sem, 1)
                nc.scalar.copy(p_sl(t), p32_sl(t)).then_inc(act_sem, 1)
                for j in range(JC):
                    if t == 0 and j == 0:
                        scalar.wait_ge(bsem, 16)
                    scalar.wait_ge(pe_sem, pe_i[("gate", t, j)])
                    if j >= 2:
                        scalar.wait_ge(dve_sem, dve_i[("mul", t, j - 2)])
                    elif t >= 1:
                        scalar.wait_ge(dve_sem, dve_i[("mul", t - 1, j + JC - 2)])
                    nc.scalar.activation(
                        g_sl(j), gate_sl(j), ACTF.Sigmoid,
                        bias=bias_sb.ap()[:, j:j + 1]).then_inc(act_sem, 1)


        # ---- DVE: max-reduce + elementwise epilogue ----
        @block.vector
        def _(vector):
            for t in range(NT):
                vector.wait_ge(act_sem, act_i[("psig", t)])
                nc.vector.tensor_reduce(
                    mlc_sb.ap()[:, t:t + 1], p32_sl(t), axis=AX.X,
                    op=ALU.max).then_inc(dve_sem, 1)
                if t >= 2:
                    vector.wait_ge(osem[t % 2], 16 * (t // 2))
                vector.wait_ge(xfsem[t % 2], 16 * (t // 2 + 1))
                for j in range(JC):
                    vector.wait_ge(act_sem, act_i[("gsig", t, j)])
                    vector.wait_ge(pe_sem, pe_i[("r", t, j)])
                    nc.vector.tensor_mul(
                        zo_sl(t, j), g_sl(j), r_sl(j)).then_inc(dve_sem, 1)
                    nc.vector.tensor_add(
                        zo_sl(t, j), zo_sl(t, j),
                        xf_sl(t, j)).then_inc(dve_sem, 1)

    return nc


def _get_program():
    if "nc" not in _prog_cache:
        _prog_cache["nc"] = _build_program()
    return _prog_cache["nc"]


def kernel(z_fused, disease_knowledge, gate_w, gate_b):
    z_fused = np.asarray(z_fused, dtype=np.float32)
    dk = np.asarray(disease_knowledge, dtype=np.float32)
    gate_w = np.asarray(gate_w, dtype=np.float32)
    gate_b = np.asarray(gate_b, dtype=np.float32)

    m0 = dk[:, 0, :]                                    # [14, D]
    m1 = dk[:, 1, :]                                    # [14, D]
    wdT = np.ascontiguousarray((m1 - m0).T).astype(NP_BF16)     # [D, 14]
    wg1T = np.ascontiguousarray(gate_w[:, :D].T).astype(NP_BF16)  # [D, D]
    wg2pT = (m1 @ gate_w[:, D:].T).astype(NP_BF16)              # [14, D]
    mpres = m1.astype(NP_BF16)                                  # [14, D]
    biasd = np.ascontiguousarray(gate_b.reshape(JC, 128).T)     # [128, JC]

    nc = _get_program()
    in_maps = []
    for c in range(CORES):
        xs = np.ascontiguousarray(
            z_fused[c * B_LOC:(c + 1) * B_LOC].reshape(T, D).T)  # [D, T]
        in_maps.append({
            "xb16": xs.astype(NP_BF16), "xf32": xs,
            "wg1T": wg1T, "wdT": wdT, "wg2pT": wg2pT, "mpres": mpres,
            "biasd": biasd,
        })

    res = run_bass_kernel_spmd(nc, in_maps, list(range(CORES)))

    z_out = np.empty((B, S, D), dtype=np.float32)
    mlc = np.empty((B, ND), dtype=np.float32)
    for c in range(CORES):
        r = res.results[c]
        z_out[c * B_LOC:(c + 1) * B_LOC] = np.ascontiguousarray(
            r["zoT"].T).reshape(B_LOC, S, D)
        m8 = r["mlc8"]                      # [14, NT] max per 512-token tile
        per_b = m8.reshape(ND, B_LOC, NT // B_LOC).max(axis=2)  # [14, 4]
        mlc[c * B_LOC:(c + 1) * B_LOC] = per_b.T
    return z_out, mlc


# revision 17
# speedup vs baseline: 1.0572x; 1.0572x over previous
"""Trainium2 Bass kernel for DiseaseKnowledgeModule.

Math (per token z in R^d, d=1024, 14 diseases x 2 states):
  score = z @ flat_memory.T / sqrt(d)                    [28]
  p     = softmax over states [:, 1] == sigmoid(s1 - s0) [14]
  mlc   = max over patches of p                          [B, 14]
  R     = p @ M_present                                  [d]
  gate  = sigmoid([z, R] @ gate_w.T + gate_b)            [d]
  z_out = z + gate * R

Device formulation (feature-major / transposed activations):
  sd    = Wdiff contraction: (m1 - m0) . z               one matmul, K=d
  p     = sigmoid(sd / sqrt(d))
  gate^T= sigmoid(Wg1 @ z^T + (M1 @ Wg2^T)^T-fold @ p^T + b)   K=d + K=14
  R^T   = M1-contraction: K=14
  zout^T= z^T + gate^T * R^T

Sharding: data-parallel over batch B=32 across 8 cores (4 batches/core).
Matmuls in bf16 (fp32 PSUM accumulate); residual add in fp32.
"""
import sys
import numpy as np

sys.path.insert(0, "/opt/trn_rl_repo")

import ml_dtypes
import concourse.bass as bass
import concourse.mybir as mybir
from concourse.bass_utils import run_bass_kernel_spmd

F32 = mybir.dt.float32
BF16 = mybir.dt.bfloat16
AX = mybir.AxisListType
ALU = mybir.AluOpType
ACTF = mybir.ActivationFunctionType
NP_BF16 = ml_dtypes.bfloat16

B, S, D = 32, 1024, 1024
ND, NS = 14, 2
CORES = 8
B_LOC = B // CORES            # 4 batches per core
T = B_LOC * S                 # 4096 tokens per core
NTILE = 512                   # tokens per T-tile (one PSUM bank of fp32)
NT = T // NTILE               # 8 T-tiles
KC = D // 128                 # 8 contraction chunks of 128
JC = D // 128                 # 8 output d-tiles of 128
XBUF = 3                      # bf16 x tile buffers
SCALE = 1.0 / float(np.sqrt(np.float32(D)))

_prog_cache = {}


def _op_numbering():
    """Per-engine op indices (1-based cumulative semaphore values)."""
    pe, act, dve = {}, {}, {}
    n = 0
    n += 1; pe[("score", 0)] = n
    for t in range(NT):
        if 2 <= t + 1 < NT:
            n += 1; pe[("score", t + 1)] = n
        for j in range(JC):
            n += 1; pe[("gate", t, j)] = n
            n += 1; pe[("r", t, j)] = n
        if t == 0 and NT > 1:
            n += 1; pe[("score", 1)] = n
    n = 0
    n += 1; act[("psig", 0)] = n
    n += 1; act[("pcast", 0)] = n
    for t in range(NT):
        for j in range(JC - 2):
            n += 1; act[("gsig", t, j)] = n
        if t + 1 < NT:
            n += 1; act[("psig", t + 1)] = n
            n += 1; act[("pcast", t + 1)] = n
        for j in range(JC - 2, JC):
            n += 1; act[("gsig", t, j)] = n
    n = 0
    for t in range(NT):
        n += 1; dve[("rmax", t)] = n
        for j in range(JC):
            n += 1; dve[("mul", t, j)] = n
            n += 1; dve[("add", t, j)] = n
    return pe, act, dve


def _build_program():
    from concourse.bass import compact_to_ranges

    pe_i, act_i, dve_i = _op_numbering()
    nc = bass.Bass()

    # A previous NEFF on this core (e.g. an XLA graph) may leave kernel-range
    # semaphores nonzero; our waits use absolute values and assume they start
    # at 0. Mirror the target_bir_lowering=True prologue: clear the whole
    # kernel sem range on gpsimd, then hold every engine at an NRT-level
    # barrier (which does not depend on bass sems) until the clear lands.
    for sem_range in compact_to_ranges(
            [s for s in nc._kernel_sem_range if s not in nc.barrier_sems]):
        nc.gpsimd.dma_reset(sem_range)
        nc.gpsimd.sem_clear(sem_range)
    nc._nrt_pseudo_barrier()

    xb16 = nc.dram_tensor("xb16", [D, T], BF16, kind="ExternalInput")
    xf32 = nc.dram_tensor("xf32", [D, T], F32, kind="ExternalInput")
    wg1T = nc.dram_tensor("wg1T", [D, D], BF16, kind="ExternalInput")
    wdT = nc.dram_tensor("wdT", [D, ND], BF16, kind="ExternalInput")
    wg2pT = nc.dram_tensor("wg2pT", [ND, D], BF16, kind="ExternalInput")
    mpres = nc.dram_tensor("mpres", [ND, D], BF16, kind="ExternalInput")
    biasd = nc.dram_tensor("biasd", [128, JC], F32, kind="ExternalInput")
    zoT = nc.dram_tensor("zoT", [D, T], F32, kind="ExternalOutput")
    mlc8 = nc.dram_tensor("mlc8", [ND, NT], F32, kind="ExternalOutput")

    from contextlib import ExitStack
    with ExitStack() as ctx:
        xb = ctx.enter_context(nc.sbuf_tensor("xb", [128, XBUF * KC * NTILE], BF16))
        xf = ctx.enter_context(nc.sbuf_tensor("xf", [128, 2 * KC * NTILE], F32))
        wg1 = ctx.enter_context(nc.sbuf_tensor("wg1", [128, KC * D], BF16))
        wdiff = ctx.enter_context(nc.sbuf_tensor("wdiff", [128, KC * ND], BF16))
        wg2p = ctx.enter_context(nc.sbuf_tensor("wg2p", [ND, D], BF16))
        mpr = ctx.enter_context(nc.sbuf_tensor("mpr", [ND, D], BF16))
        bias_sb = ctx.enter_context(nc.sbuf_tensor("bias_sb", [128, JC], F32))
        zbias = ctx.enter_context(nc.sbuf_tensor("zbias", [ND, 1], F32))
        p_sb = ctx.enter_context(nc.sbuf_tensor("p_sb", [ND, 2 * NTILE], BF16))
        g_sb = ctx.enter_context(nc.sbuf_tensor("g_sb", [128, 2 * NTILE], F32))
        zo = ctx.enter_context(nc.sbuf_tensor("zo", [128, 2 * JC * NTILE], F32))
        p32_sb = ctx.enter_context(nc.sbuf_tensor("p32_sb", [ND, 2 * NTILE], F32))
        mlc_sb = ctx.enter_context(nc.sbuf_tensor("mlc_sb", [ND, NT], F32))
        sd_ps = ctx.enter_context(nc.psum_tensor("sd_ps", [ND, 2 * NTILE], F32))
        gate_ps = ctx.enter_context(nc.psum_tensor("gate_ps", [128, 2 * NTILE], F32))
        r_ps = ctx.enter_context(nc.psum_tensor("r_ps", [128, 2 * NTILE], F32))
        zsem = ctx.enter_context(nc.semaphore("zsem"))
        wdsem = ctx.enter_context(nc.semaphore("wdsem"))
        bsem = ctx.enter_context(nc.semaphore("bsem"))
        wsem = ctx.enter_context(nc.semaphore("wsem"))
        w1sem = [ctx.enter_context(nc.semaphore(f"w1sem{i}")) for i in range(4)]
        xbsem = [ctx.enter_context(nc.semaphore(f"xbsem{i}"))
                 for i in range(XBUF)]
        xfsem = [ctx.enter_context(nc.semaphore(f"xfsem{i}"))
                 for i in range(2)]
        osem = [ctx.enter_context(nc.semaphore(f"osem{i}"))
                for i in range(2)]
        pe_sem = ctx.enter_context(nc.semaphore("pe_sem"))
        act_sem = ctx.enter_context(nc.semaphore("act_sem"))
        dve_sem = ctx.enter_context(nc.semaphore("dve_sem"))
        block = ctx.enter_context(nc.Block(no_gpsimd_drain=True))

        # ---- slice helpers ----
        def xb_sl(t, k):        # bf16 rhs [128, NTILE] for k-chunk of tile t
            o = (t % XBUF) * KC * NTILE + k * NTILE
            return xb.ap()[:, o:o + NTILE]

        def xb_tile(t):
            o = (t % XBUF) * KC * NTILE
            return xb.ap()[:, o:o + KC * NTILE].rearrange(
                "p (k n) -> p k n", k=KC)

        def xf_sl(t, j):        # f32 residual [128, NTILE] for d-tile j
            o = (t % 2) * KC * NTILE + j * NTILE
            return xf.ap()[:, o:o + NTILE]

        def xf_tile(t):
            o = (t % 2) * KC * NTILE
            return xf.ap()[:, o:o + KC * NTILE].rearrange(
                "p (k n) -> p k n", k=KC)

        def wg1_sl(k, j):
            o = k * D + j * 128
            return wg1.ap()[:, o:o + 128]

        def wd_sl(k):
            o = k * ND
            return wdiff.ap()[:, o:o + ND]

        def p_sl(t):
            o = (t % 2) * NTILE
            return p_sb.ap()[:, o:o + NTILE]

        def p32_sl(t):
            o = (t % 2) * NTILE
            return p32_sb.ap()[:, o:o + NTILE]

        def g_sl(j):
            o = (j % 2) * NTILE
            return g_sb.ap()[:, o:o + NTILE]

        def zo_sl(t, j):
            o = (t % 2) * JC * NTILE + j * NTILE
            return zo.ap()[:, o:o + NTILE]

        def zo_tile(t):
            o = (t % 2) * JC * NTILE
            return zo.ap()[:, o:o + JC * NTILE].rearrange(
                "p (j n) -> p j n", j=JC)

        def sd_sl(t):
            o = (t % 2) * NTILE
            return sd_ps.ap()[:, o:o + NTILE]

        def gate_sl(j):
            o = (j % 2) * NTILE
            return gate_ps.ap()[:, o:o + NTILE]

        def r_sl(j):
            o = (j % 2) * NTILE
            return r_ps.ap()[:, o:o + NTILE]

        # ---- GPSIMD: constants init ----
        @block.gpsimd
        def _(gpsimd):
            nc.gpsimd.memset(zbias.ap(), 0.0).then_inc(zsem, 1)

        # ---- SP: all DMA ----
        @block.sync
        def _(sync):
            xbsrc = xb16.rearrange("(k p) (t n) -> p k t n", p=128, n=NTILE)
            xfsrc = xf32.rearrange("(k p) (t n) -> p k t n", p=128, n=NTILE)

            sync.dma_start(
                wdiff.ap().rearrange("p (k m) -> p k m", k=KC),
                wdT.rearrange("(k p) m -> p k m", p=128),
            ).then_inc(wdsem, 16)
            sync.dma_start(bias_sb.ap(), biasd[:, :]).then_inc(bsem, 16)
            sync.dma_start(xb_tile(0), xbsrc[:, :, 0, :]).then_inc(xbsem[0], 16)
            w1dst = wg1.ap().rearrange("p (k m) -> p k m", k=KC)
            w1src = wg1T.rearrange("(k p) m -> p k m", p=128)
            for q in range(4):
                sync.dma_start(w1dst[:, 2 * q:2 * q + 2, :],
                               w1src[:, 2 * q:2 * q + 2, :]
                               ).then_inc(w1sem[q], 16)
            sync.dma_start(wg2p.ap(), wg2pT[:, :]).then_inc(wsem, 16)
            sync.dma_start(mpr.ap(), mpres[:, :]).then_inc(wsem, 16)
            sync.dma_start(xb_tile(1), xbsrc[:, :, 1, :]).then_inc(xbsem[1], 16)
            sync.dma_start(xf_tile(0), xfsrc[:, :, 0, :]).then_inc(xfsem[0], 16)
            sync.dma_start(xb_tile(2), xbsrc[:, :, 2, :]).then_inc(xbsem[2], 16)
            sync.dma_start(xf_tile(1), xfsrc[:, :, 1, :]).then_inc(xfsem[1], 16)

            zdst = zoT.rearrange("(j p) (t n) -> p j t n", p=128, n=NTILE)
            for t in range(NT - 1):
                sync.wait_ge(dve_sem, dve_i[("add", t, JC - 1)])
                sync.dma_start(zdst[:, :, t, :],
                               zo_tile(t)).then_inc(osem[t % 2], 16)
                if t + XBUF < NT:
                    sync.dma_start(xb_tile(t + XBUF),
                                   xbsrc[:, :, t + XBUF, :]
                                   ).then_inc(xbsem[t % XBUF], 16)
                if t + 2 < NT:
                    sync.dma_start(xf_tile(t + 2),
                                   xfsrc[:, :, t + 2, :]
                                   ).then_inc(xfsem[t % 2], 16)
            # last tile: stream each d-slice out as soon as its add lands,
            # and let the (tiny) mlc transfer fly as early as possible
            tl = NT - 1
            sync.wait_ge(dve_sem, dve_i[("rmax", NT - 1)])
            sync.dma_start(mlc8[:, :], mlc_sb.ap()).then_inc(osem[0], 16)
            for j in range(JC):
                sync.wait_ge(dve_sem, dve_i[("add", tl, j)])
                sync.dma_start(zdst[:, j, tl, :],
                               zo_sl(tl, j)).then_inc(osem[tl % 2], 16)
            # slot0: tiles 0,2,4,6 + mlc = 5 transfers; slot1: 1,3,5 + 8 j
            sync.wait_ge(osem[0], 16 * (NT // 2 + 1))
            sync.wait_ge(osem[1], 16 * (NT // 2 - 1 + JC))

        # ---- PE: all matmuls (bf16 in, fp32 accumulate) ----
        @block.tensor
        def _(tensor):
            def score(t):
                tensor.wait_ge(xbsem[t % XBUF], 16 * (t // XBUF + 1))
                if t >= 2:
                    tensor.wait_ge(act_sem, act_i[("psig", t - 2)])
                for k in range(KC):
                    mm = nc.tensor.matmul(
                        sd_sl(t), wd_sl(k), xb_sl(t, k),
                        start=(k == 0), stop=(k == KC - 1))
                mm.then_inc(pe_sem, 1)

            tensor.wait_ge(wdsem, 16)
            score(0)
            for t in range(NT):
                if 2 <= t + 1 < NT:
                    score(t + 1)
                tensor.wait_ge(act_sem, act_i[("pcast", t)])
                for j in range(JC):
                    # gate_pre_j = sum_k Wg1[k,j]^T x[k] + Wg2p[j]^T p
                    if j >= 2:
                        tensor.wait_ge(act_sem, act_i[("gsig", t, j - 2)])
                    elif t >= 1:
                        tensor.wait_ge(act_sem, act_i[("gsig", t - 1, j + JC - 2)])
                    for k in range(KC):
                        if t == 0 and j == 0:
                            if k % 2 == 0:
                                tensor.wait_ge(w1sem[k // 2], 16)
                            if k == KC - 1:
                                tensor.wait_ge(wsem, 32)
                        nc.tensor.matmul(
                            gate_sl(j), wg1_sl(k, j), xb_sl(t, k),
                            start=(k == 0), stop=False)
                    mm = nc.tensor.matmul(
                        gate_sl(j), wg2p.ap()[:, j * 128:(j + 1) * 128],
                        p_sl(t), start=False, stop=True)
                    mm.then_inc(pe_sem, 1)
                    # R_j = M1[j]^T p
                    if j >= 2:
                        tensor.wait_ge(dve_sem, dve_i[("mul", t, j - 2)])
                    elif t >= 1:
                        tensor.wait_ge(dve_sem, dve_i[("mul", t - 1, j + JC - 2)])
                    mm = nc.tensor.matmul(
                        r_sl(j), mpr.ap()[:, j * 128:(j + 1) * 128],
                        p_sl(t), start=True, stop=True)
                    mm.then_inc(pe_sem, 1)
                if t == 0 and NT > 1:
                    score(1)

        # ---- ACT: sigmoids ----
        @block.scalar
        def _(scalar):
            def psig(t):
                scalar.wait_ge(pe_sem, pe_i[("score", t)])
                if t >= 2:
                    scalar.wait_ge(pe_sem, pe_i[("r", t - 2, JC - 1)])
                    scalar.wait_ge(dve_sem, dve_i[("rmax", t - 2)])
                nc.scalar.activation(
                    p32_sl(t), sd_sl(t), ACTF.Sigmoid, bias=zbias.ap(),
                    scale=SCALE).then_inc(act_sem, 1)
                nc.scalar.copy(p_sl(t), p32_sl(t)).then_inc(act_sem, 1)

            scalar.wait_ge(zsem, 1)
            psig(0)
            for t in range(NT):
                for j in range(JC):
                    if t == 0 and j == 0:
                        scalar.wait_ge(bsem, 16)
                    if j == JC - 2 and t + 1 < NT:
                        psig(t + 1)
                    scalar.wait_ge(pe_sem, pe_i[("gate", t, j)])
                    if j >= 2:
                        scalar.wait_ge(dve_sem, dve_i[("mul", t, j - 2)])
                    elif t >= 1:
                        scalar.wait_ge(dve_sem, dve_i[("mul", t - 1, j + JC - 2)])
                    nc.scalar.activation(
                        g_sl(j), gate_sl(j), ACTF.Sigmoid,
                        bias=bias_sb.ap()[:, j:j + 1]).then_inc(act_sem, 1)


        # ---- DVE: max-reduce + elementwise epilogue ----
        @block.vector
        def _(vector):
            for t in range(NT):
                vector.wait_ge(act_sem, act_i[("psig", t)])
                nc.vector.tensor_reduce(
                    mlc_sb.ap()[:, t:t + 1], p32_sl(t), axis=AX.X,
                    op=ALU.max).then_inc(dve_sem, 1)
                if t >= 2:
                    vector.wait_ge(osem[t % 2], 16 * (t // 2))
                vector.wait_ge(xfsem[t % 2], 16 * (t // 2 + 1))
                for j in range(JC):
                    vector.wait_ge(act_sem, act_i[("gsig", t, j)])
                    vector.wait_ge(pe_sem, pe_i[("r", t, j)])
                    nc.vector.tensor_mul(
                        zo_sl(t, j), g_sl(j), r_sl(j)).then_inc(dve_sem, 1)
                    nc.vector.tensor_add(
                        zo_sl(t, j), zo_sl(t, j),
                        xf_sl(t, j)).then_inc(dve_sem, 1)

    return nc


def _get_program():
    if "nc" not in _prog_cache:
        _prog_cache["nc"] = _build_program()
    return _prog_cache["nc"]


def kernel(z_fused, disease_knowledge, gate_w, gate_b):
    z_fused = np.asarray(z_fused, dtype=np.float32)
    dk = np.asarray(disease_knowledge, dtype=np.float32)
    gate_w = np.asarray(gate_w, dtype=np.float32)
    gate_b = np.asarray(gate_b, dtype=np.float32)

    m0 = dk[:, 0, :]                                    # [14, D]
    m1 = dk[:, 1, :]                                    # [14, D]
    wdT = np.ascontiguousarray((m1 - m0).T).astype(NP_BF16)     # [D, 14]
    wg1T = np.ascontiguousarray(gate_w[:, :D].T).astype(NP_BF16)  # [D, D]
    wg2pT = (m1 @ gate_w[:, D:].T).astype(NP_BF16)              # [14, D]
    mpres = m1.astype(NP_BF16)                                  # [14, D]
    biasd = np.ascontiguousarray(gate_b.reshape(JC, 128).T)     # [128, JC]

    nc = _get_program()
    in_maps = []
    for c in range(CORES):
        xs = np.ascontiguousarray(
            z_fused[c * B_LOC:(c + 1) * B_LOC].reshape(T, D).T)  # [D, T]
        in_maps.append({
            "xb16": xs.astype(NP_BF16), "xf32": xs,
            "wg1T": wg1T, "wdT": wdT, "wg2pT": wg2pT, "mpres": mpres,
            "biasd": biasd,
        })

    res = run_bass_kernel_spmd(nc, in_maps, list(range(CORES)))

    z_out = np.empty((B, S, D), dtype=np.float32)
    mlc = np.empty((B, ND), dtype=np.float32)
    for c in range(CORES):
        r = res.results[c]
        z_out[c * B_LOC:(c + 1) * B_LOC] = np.ascontiguousarray(
            r["zoT"].T).reshape(B_LOC, S, D)
        m8 = r["mlc8"]                      # [14, NT] max per 512-token tile
        per_b = m8.reshape(ND, B_LOC, NT // B_LOC).max(axis=2)  # [14, 4]
        mlc[c * B_LOC:(c + 1) * B_LOC] = per_b.T
    return z_out, mlc


# revision 18
# speedup vs baseline: 1.0575x; 1.0002x over previous
"""Trainium2 Bass kernel for DiseaseKnowledgeModule.

Math (per token z in R^d, d=1024, 14 diseases x 2 states):
  score = z @ flat_memory.T / sqrt(d)                    [28]
  p     = softmax over states [:, 1] == sigmoid(s1 - s0) [14]
  mlc   = max over patches of p                          [B, 14]
  R     = p @ M_present                                  [d]
  gate  = sigmoid([z, R] @ gate_w.T + gate_b)            [d]
  z_out = z + gate * R

Device formulation (feature-major / transposed activations):
  sd    = Wdiff contraction: (m1 - m0) . z               one matmul, K=d
  p     = sigmoid(sd / sqrt(d))
  gate^T= sigmoid(Wg1 @ z^T + (M1 @ Wg2^T)^T-fold @ p^T + b)   K=d + K=14
  R^T   = M1-contraction: K=14
  zout^T= z^T + gate^T * R^T

Sharding: data-parallel over batch B=32 across 8 cores (4 batches/core).
Matmuls in bf16 (fp32 PSUM accumulate); residual add in fp32.
"""
import sys
import numpy as np

sys.path.insert(0, "/opt/trn_rl_repo")

import ml_dtypes
import concourse.bass as bass
import concourse.mybir as mybir
from concourse.bass_utils import run_bass_kernel_spmd

F32 = mybir.dt.float32
BF16 = mybir.dt.bfloat16
AX = mybir.AxisListType
ALU = mybir.AluOpType
ACTF = mybir.ActivationFunctionType
NP_BF16 = ml_dtypes.bfloat16

B, S, D = 32, 1024, 1024
ND, NS = 14, 2
CORES = 8
B_LOC = B // CORES            # 4 batches per core
T = B_LOC * S                 # 4096 tokens per core
NTILE = 512                   # tokens per T-tile (one PSUM bank of fp32)
NT = T // NTILE               # 8 T-tiles
KC = D // 128                 # 8 contraction chunks of 128
JC = D // 128                 # 8 output d-tiles of 128
XBUF = 3                      # bf16 x tile buffers
SCALE = 1.0 / float(np.sqrt(np.float32(D)))

_prog_cache = {}


def _op_numbering():
    """Per-engine op indices (1-based cumulative semaphore values)."""
    pe, act, dve = {}, {}, {}
    n = 0
    n += 1; pe[("score", 0)] = n
    for t in range(NT):
        if 2 <= t + 1 < NT:
            n += 1; pe[("score", t + 1)] = n
        for j in range(JC):
            n += 1; pe[("gate", t, j)] = n
            n += 1; pe[("r", t, j)] = n
        if t == 0 and NT > 1:
            n += 1; pe[("score", 1)] = n
    n = 0
    n += 1; act[("psig", 0)] = n
    n += 1; act[("pcast", 0)] = n
    for t in range(NT):
        for j in range(JC - 2):
            n += 1; act[("gsig", t, j)] = n
        if t + 1 < NT:
            n += 1; act[("psig", t + 1)] = n
            n += 1; act[("pcast", t + 1)] = n
        for j in range(JC - 2, JC):
            n += 1; act[("gsig", t, j)] = n
    n = 0
    for t in range(NT):
        n += 1; dve[("rmax", t)] = n
        for j in range(JC):
            n += 1; dve[("mul", t, j)] = n
            n += 1; dve[("add", t, j)] = n
    return pe, act, dve


def _build_program():
    from concourse.bass import compact_to_ranges

    pe_i, act_i, dve_i = _op_numbering()
    nc = bass.Bass()

    # A previous NEFF on this core (e.g. an XLA graph) may leave kernel-range
    # semaphores nonzero; our waits use absolute values and assume they start
    # at 0. Mirror the target_bir_lowering=True prologue: clear the whole
    # kernel sem range on gpsimd, then hold every engine at an NRT-level
    # barrier (which does not depend on bass sems) until the clear lands.
    # only ids up to ~172 are allocated (data + queue sems); clearing the
    # full 150-255 range costs several extra us of serialized prologue.
    for sem_range in compact_to_ranges(
            [s for s in range(150, 192) if s not in nc.barrier_sems]):
        nc.gpsimd.dma_reset(sem_range)
        nc.gpsimd.sem_clear(sem_range)
    nc._nrt_pseudo_barrier()

    xb16 = nc.dram_tensor("xb16", [D, T], BF16, kind="ExternalInput")
    xf32 = nc.dram_tensor("xf32", [D, T], F32, kind="ExternalInput")
    wg1T = nc.dram_tensor("wg1T", [D, D], BF16, kind="ExternalInput")
    wdT = nc.dram_tensor("wdT", [D, ND], BF16, kind="ExternalInput")
    wg2pT = nc.dram_tensor("wg2pT", [ND, D], BF16, kind="ExternalInput")
    mpres = nc.dram_tensor("mpres", [ND, D], BF16, kind="ExternalInput")
    biasd = nc.dram_tensor("biasd", [128, JC], F32, kind="ExternalInput")
    zoT = nc.dram_tensor("zoT", [D, T], F32, kind="ExternalOutput")
    mlc8 = nc.dram_tensor("mlc8", [ND, NT], F32, kind="ExternalOutput")

    from contextlib import ExitStack
    with ExitStack() as ctx:
        xb = ctx.enter_context(nc.sbuf_tensor("xb", [128, XBUF * KC * NTILE], BF16))
        xf = ctx.enter_context(nc.sbuf_tensor("xf", [128, 2 * KC * NTILE], F32))
        wg1 = ctx.enter_context(nc.sbuf_tensor("wg1", [128, KC * D], BF16))
        wdiff = ctx.enter_context(nc.sbuf_tensor("wdiff", [128, KC * ND], BF16))
        wg2p = ctx.enter_context(nc.sbuf_tensor("wg2p", [ND, D], BF16))
        mpr = ctx.enter_context(nc.sbuf_tensor("mpr", [ND, D], BF16))
        bias_sb = ctx.enter_context(nc.sbuf_tensor("bias_sb", [128, JC], F32))
        zbias = ctx.enter_context(nc.sbuf_tensor("zbias", [ND, 1], F32))
        p_sb = ctx.enter_context(nc.sbuf_tensor("p_sb", [ND, 2 * NTILE], BF16))
        g_sb = ctx.enter_context(nc.sbuf_tensor("g_sb", [128, 2 * NTILE], F32))
        zo = ctx.enter_context(nc.sbuf_tensor("zo", [128, 2 * JC * NTILE], F32))
        p32_sb = ctx.enter_context(nc.sbuf_tensor("p32_sb", [ND, 2 * NTILE], F32))
        mlc_sb = ctx.enter_context(nc.sbuf_tensor("mlc_sb", [ND, NT], F32))
        sd_ps = ctx.enter_context(nc.psum_tensor("sd_ps", [ND, 2 * NTILE], F32))
        gate_ps = ctx.enter_context(nc.psum_tensor("gate_ps", [128, 2 * NTILE], F32))
        r_ps = ctx.enter_context(nc.psum_tensor("r_ps", [128, 2 * NTILE], F32))
        zsem = ctx.enter_context(nc.semaphore("zsem"))
        wdsem = ctx.enter_context(nc.semaphore("wdsem"))
        bsem = ctx.enter_context(nc.semaphore("bsem"))
        wsem = ctx.enter_context(nc.semaphore("wsem"))
        w1sem = [ctx.enter_context(nc.semaphore(f"w1sem{i}")) for i in range(4)]
        xbsem = [ctx.enter_context(nc.semaphore(f"xbsem{i}"))
                 for i in range(XBUF)]
        xfsem = [ctx.enter_context(nc.semaphore(f"xfsem{i}"))
                 for i in range(2)]
        osem = [ctx.enter_context(nc.semaphore(f"osem{i}"))
                for i in range(2)]
        pe_sem = ctx.enter_context(nc.semaphore("pe_sem"))
        act_sem = ctx.enter_context(nc.semaphore("act_sem"))
        dve_sem = ctx.enter_context(nc.semaphore("dve_sem"))
        block = ctx.enter_context(nc.Block(no_gpsimd_drain=True))

        # ---- slice helpers ----
        def xb_sl(t, k):        # bf16 rhs [128, NTILE] for k-chunk of tile t
            o = (t % XBUF) * KC * NTILE + k * NTILE
            return xb.ap()[:, o:o + NTILE]

        def xb_tile(t):
            o = (t % XBUF) * KC * NTILE
            return xb.ap()[:, o:o + KC * NTILE].rearrange(
                "p (k n) -> p k n", k=KC)

        def xf_sl(t, j):        # f32 residual [128, NTILE] for d-tile j
            o = (t % 2) * KC * NTILE + j * NTILE
            return xf.ap()[:, o:o + NTILE]

        def xf_tile(t):
            o = (t % 2) * KC * NTILE
            return xf.ap()[:, o:o + KC * NTILE].rearrange(
                "p (k n) -> p k n", k=KC)

        def wg1_sl(k, j):
            o = k * D + j * 128
            return wg1.ap()[:, o:o + 128]

        def wd_sl(k):
            o = k * ND
            return wdiff.ap()[:, o:o + ND]

        def p_sl(t):
            o = (t % 2) * NTILE
            return p_sb.ap()[:, o:o + NTILE]

        def p32_sl(t):
            o = (t % 2) * NTILE
            return p32_sb.ap()[:, o:o + NTILE]

        def g_sl(j):
            o = (j % 2) * NTILE
            return g_sb.ap()[:, o:o + NTILE]

        def zo_sl(t, j):
            o = (t % 2) * JC * NTILE + j * NTILE
            return zo.ap()[:, o:o + NTILE]

        def zo_tile(t):
            o = (t % 2) * JC * NTILE
            return zo.ap()[:, o:o + JC * NTILE].rearrange(
                "p (j n) -> p j n", j=JC)

        def sd_sl(t):
            o = (t % 2) * NTILE
            return sd_ps.ap()[:, o:o + NTILE]

        def gate_sl(j):
            o = (j % 2) * NTILE
            return gate_ps.ap()[:, o:o + NTILE]

        def r_sl(j):
            o = (j % 2) * NTILE
            return r_ps.ap()[:, o:o + NTILE]

        # ---- GPSIMD: constants init ----
        @block.gpsimd
        def _(gpsimd):
            nc.gpsimd.memset(zbias.ap(), 0.0).then_inc(zsem, 1)

        # ---- SP: all DMA ----
        @block.sync
        def _(sync):
            xbsrc = xb16.rearrange("(k p) (t n) -> p k t n", p=128, n=NTILE)
            xfsrc = xf32.rearrange("(k p) (t n) -> p k t n", p=128, n=NTILE)

            sync.dma_start(
                wdiff.ap().rearrange("p (k m) -> p k m", k=KC),
                wdT.rearrange("(k p) m -> p k m", p=128),
            ).then_inc(wdsem, 16)
            sync.dma_start(bias_sb.ap(), biasd[:, :]).then_inc(bsem, 16)
            sync.dma_start(xb_tile(0), xbsrc[:, :, 0, :]).then_inc(xbsem[0], 16)
            w1dst = wg1.ap().rearrange("p (k m) -> p k m", k=KC)
            w1src = wg1T.rearrange("(k p) m -> p k m", p=128)
            for q in range(4):
                sync.dma_start(w1dst[:, 2 * q:2 * q + 2, :],
                               w1src[:, 2 * q:2 * q + 2, :]
                               ).then_inc(w1sem[q], 16)
            sync.dma_start(wg2p.ap(), wg2pT[:, :]).then_inc(wsem, 16)
            sync.dma_start(mpr.ap(), mpres[:, :]).then_inc(wsem, 16)
            sync.dma_start(xb_tile(1), xbsrc[:, :, 1, :]).then_inc(xbsem[1], 16)
            sync.dma_start(xf_tile(0), xfsrc[:, :, 0, :]).then_inc(xfsem[0], 16)
            sync.dma_start(xb_tile(2), xbsrc[:, :, 2, :]).then_inc(xbsem[2], 16)
            sync.dma_start(xf_tile(1), xfsrc[:, :, 1, :]).then_inc(xfsem[1], 16)

            zdst = zoT.rearrange("(j p) (t n) -> p j t n", p=128, n=NTILE)
            for t in range(NT - 1):
                sync.wait_ge(dve_sem, dve_i[("add", t, JC - 1)])
                sync.dma_start(zdst[:, :, t, :],
                               zo_tile(t)).then_inc(osem[t % 2], 16)
                if t + XBUF < NT:
                    sync.dma_start(xb_tile(t + XBUF),
                                   xbsrc[:, :, t + XBUF, :]
                                   ).then_inc(xbsem[t % XBUF], 16)
                if t + 2 < NT:
                    sync.dma_start(xf_tile(t + 2),
                                   xfsrc[:, :, t + 2, :]
                                   ).then_inc(xfsem[t % 2], 16)
            # last tile: stream each d-slice out as soon as its add lands,
            # and let the (tiny) mlc transfer fly as early as possible
            tl = NT - 1
            sync.wait_ge(dve_sem, dve_i[("rmax", NT - 1)])
            sync.dma_start(mlc8[:, :], mlc_sb.ap()).then_inc(osem[0], 16)
            for j in range(JC):
                sync.wait_ge(dve_sem, dve_i[("add", tl, j)])
                sync.dma_start(zdst[:, j, tl, :],
                               zo_sl(tl, j)).then_inc(osem[tl % 2], 16)
            # slot0: tiles 0,2,4,6 + mlc = 5 transfers; slot1: 1,3,5 + 8 j
            sync.wait_ge(osem[0], 16 * (NT // 2 + 1))
            sync.wait_ge(osem[1], 16 * (NT // 2 - 1 + JC))

        # ---- PE: all matmuls (bf16 in, fp32 accumulate) ----
        @block.tensor
        def _(tensor):
            def score(t):
                tensor.wait_ge(xbsem[t % XBUF], 16 * (t // XBUF + 1))
                if t >= 2:
                    tensor.wait_ge(act_sem, act_i[("psig", t - 2)])
                for k in range(KC):
                    mm = nc.tensor.matmul(
                        sd_sl(t), wd_sl(k), xb_sl(t, k),
                        start=(k == 0), stop=(k == KC - 1))
                mm.then_inc(pe_sem, 1)

            tensor.wait_ge(wdsem, 16)
            score(0)
            for t in range(NT):
                if 2 <= t + 1 < NT:
                    score(t + 1)
                tensor.wait_ge(act_sem, act_i[("pcast", t)])
                for j in range(JC):
                    # gate_pre_j = sum_k Wg1[k,j]^T x[k] + Wg2p[j]^T p
                    if j >= 2:
                        tensor.wait_ge(act_sem, act_i[("gsig", t, j - 2)])
                    elif t >= 1:
                        tensor.wait_ge(act_sem, act_i[("gsig", t - 1, j + JC - 2)])
                    for k in range(KC):
                        if t == 0 and j == 0:
                            if k % 2 == 0:
                                tensor.wait_ge(w1sem[k // 2], 16)
                            if k == KC - 1:
                                tensor.wait_ge(wsem, 32)
                        nc.tensor.matmul(
                            gate_sl(j), wg1_sl(k, j), xb_sl(t, k),
                            start=(k == 0), stop=False)
                    mm = nc.tensor.matmul(
                        gate_sl(j), wg2p.ap()[:, j * 128:(j + 1) * 128],
                        p_sl(t), start=False, stop=True)
                    mm.then_inc(pe_sem, 1)
                    # R_j = M1[j]^T p
                    if j >= 2:
                        tensor.wait_ge(dve_sem, dve_i[("mul", t, j - 2)])
                    elif t >= 1:
                        tensor.wait_ge(dve_sem, dve_i[("mul", t - 1, j + JC - 2)])
                    mm = nc.tensor.matmul(
                        r_sl(j), mpr.ap()[:, j * 128:(j + 1) * 128],
                        p_sl(t), start=True, stop=True)
                    mm.then_inc(pe_sem, 1)
                if t == 0 and NT > 1:
                    score(1)

        # ---- ACT: sigmoids ----
        @block.scalar
        def _(scalar):
            def psig(t):
                scalar.wait_ge(pe_sem, pe_i[("score", t)])
                if t >= 2:
                    scalar.wait_ge(pe_sem, pe_i[("r", t - 2, JC - 1)])
                    scalar.wait_ge(dve_sem, dve_i[("rmax", t - 2)])
                nc.scalar.activation(
                    p32_sl(t), sd_sl(t), ACTF.Sigmoid, bias=zbias.ap(),
                    scale=SCALE).then_inc(act_sem, 1)
                nc.scalar.copy(p_sl(t), p32_sl(t)).then_inc(act_sem, 1)

            scalar.wait_ge(zsem, 1)
            psig(0)
            for t in range(NT):
                for j in range(JC):
                    if t == 0 and j == 0:
                        scalar.wait_ge(bsem, 16)
                    if j == JC - 2 and t + 1 < NT:
                        psig(t + 1)
                    scalar.wait_ge(pe_sem, pe_i[("gate", t, j)])
                    if j >= 2:
                        scalar.wait_ge(dve_sem, dve_i[("mul", t, j - 2)])
                    elif t >= 1:
                        scalar.wait_ge(dve_sem, dve_i[("mul", t - 1, j + JC - 2)])
                    nc.scalar.activation(
                        g_sl(j), gate_sl(j), ACTF.Sigmoid,
                        bias=bias_sb.ap()[:, j:j + 1]).then_inc(act_sem, 1)


        # ---- DVE: max-reduce + elementwise epilogue ----
        @block.vector
        def _(vector):
            for t in range(NT):
                vector.wait_ge(act_sem, act_i[("psig", t)])
                nc.vector.tensor_reduce(
                    mlc_sb.ap()[:, t:t + 1], p32_sl(t), axis=AX.X,
                    op=ALU.max).then_inc(dve_sem, 1)
                if t >= 2:
                    vector.wait_ge(osem[t % 2], 16 * (t // 2))
                vector.wait_ge(xfsem[t % 2], 16 * (t // 2 + 1))
                for j in range(JC):
                    vector.wait_ge(act_sem, act_i[("gsig", t, j)])
                    vector.wait_ge(pe_sem, pe_i[("r", t, j)])
                    nc.vector.tensor_mul(
                        zo_sl(t, j), g_sl(j), r_sl(j)).then_inc(dve_sem, 1)
                    nc.vector.tensor_add(
                        zo_sl(t, j), zo_sl(t, j),
                        xf_sl(t, j)).then_inc(dve_sem, 1)

    return nc


def _get_program():
    if "nc" not in _prog_cache:
        _prog_cache["nc"] = _build_program()
    return _prog_cache["nc"]


def kernel(z_fused, disease_knowledge, gate_w, gate_b):
    z_fused = np.asarray(z_fused, dtype=np.float32)
    dk = np.asarray(disease_knowledge, dtype=np.float32)
    gate_w = np.asarray(gate_w, dtype=np.float32)
    gate_b = np.asarray(gate_b, dtype=np.float32)

    m0 = dk[:, 0, :]                                    # [14, D]
    m1 = dk[:, 1, :]                                    # [14, D]
    wdT = np.ascontiguousarray((m1 - m0).T).astype(NP_BF16)     # [D, 14]
    wg1T = np.ascontiguousarray(gate_w[:, :D].T).astype(NP_BF16)  # [D, D]
    wg2pT = (m1 @ gate_w[:, D:].T).astype(NP_BF16)              # [14, D]
    mpres = m1.astype(NP_BF16)                                  # [14, D]
    biasd = np.ascontiguousarray(gate_b.reshape(JC, 128).T)     # [128, JC]

    nc = _get_program()
    in_maps = []
    for c in range(CORES):
        xs = np.ascontiguousarray(
            z_fused[c * B_LOC:(c + 1) * B_LOC].reshape(T, D).T)  # [D, T]
        in_maps.append({
            "xb16": xs.astype(NP_BF16), "xf32": xs,
            "wg1T": wg1T, "wdT": wdT, "wg2pT": wg2pT, "mpres": mpres,
            "biasd": biasd,
        })

    res = run_bass_kernel_spmd(nc, in_maps, list(range(CORES)))

    z_out = np.empty((B, S, D), dtype=np.float32)
    mlc = np.empty((B, ND), dtype=np.float32)
    for c in range(CORES):
        r = res.results[c]
        z_out[c * B_LOC:(c + 1) * B_LOC] = np.ascontiguousarray(
            r["zoT"].T).reshape(B_LOC, S, D)
        m8 = r["mlc8"]                      # [14, NT] max per 512-token tile
        per_b = m8.reshape(ND, B_LOC, NT // B_LOC).max(axis=2)  # [14, 4]
        mlc[c * B_LOC:(c + 1) * B_LOC] = per_b.T
    return z_out, mlc
